# revision 1
# baseline (speedup 1.0000x reference)
"""AvgDistanceConv (GNN message passing) on 8 Trainium2 NeuronCores.

out[:, 0] = pos = h[:, 0]
out[:, 1] = segment_mean over incoming edges of |pos[src] - pos[dst]|

Strategy
--------
Shard by *destination range*: core c owns nodes [c*12500, (c+1)*12500) and
processes exactly the edges pointing into them, so each core produces its
output slice independently -- no collectives needed (better than edge
sharding + all-reduce: same gather volume, zero communication).

Host prep is index-only (cast/sort/bincount/pad); all float work runs on
device:
  * per core, build a degree-sorted padded ELL table of src indices
    (per-128-row-tile K = max in-degree in tile; pad slots hold the row's
    own node id so they contribute |pos[n]-pos[n]| = 0),
  * device gathers pos[src] via indirect DMA (128 offsets/call -- the only
    dynamic-offset granularity this DGE config supports), subtracts the
    per-partition scalar pos[dst], abs-sum-reduces each row, divides by
    max(count, 1), and emits [pos, mean] pairs.
"""
import sys
sys.path.insert(0, '/opt/trn_rl_repo')
import numpy as np
import concourse.bass as bass
import concourse.mybir as mybir
from concourse.bass_utils import run_bass_kernel_spmd
from concourse.tile import TileContext

P = 128
NC = 8
N_NODES = 100000


def _split_sync_waits(nc, max_waits=1):
    """This walrus build rejects more than one sync wait per instruction.
    Hoist extras into standalone same-engine EventSemaphore waits placed
    immediately before the owning instruction (same-engine program order
    preserves the synchronization semantics)."""
    for f in nc.m.functions:
        for blk in f.blocks:
            insts = list(blk.instructions)
            new = []
            dirty = False
            for inst in insts:
                si = inst.sync_info
                if si is not None and len(si.on_wait) > max_waits:
                    waits = list(si.on_wait)
                    for j, w in enumerate(waits[:-max_waits]):
                        wi = mybir.InstEventSemaphore(
                            name=f"{inst.name}_hw{j}", ins=[], outs=[])
                        wi.engine = inst.engine
                        wi.sync_info = mybir.SyncInfo(on_wait=[w], on_update=[])
                        new.append(wi)
                    inst.sync_info = mybir.SyncInfo(
                        on_wait=waits[-max_waits:], on_update=list(si.on_update))
                    dirty = True
                new.append(inst)
            if dirty:
                blk.instructions = new


def _host_prep(h, src, dst):
    N = N_NODES
    NPC = N // NC
    TILES = (NPC + P - 1) // P
    ROWS = TILES * P
    E = src.shape[0]

    pos = np.ascontiguousarray(h[:, 0], dtype=np.float32).reshape(N, 1)
    src32 = src.astype(np.int32)
    dst32 = dst.astype(np.int32)

    cnt = np.bincount(dst32, minlength=N)
    order = np.argsort(dst32, kind='stable')
    ssrc = src32[order]
    starts = np.zeros(N + 1, np.int64)
    starts[1:] = np.cumsum(cnt)

    deg_c = cnt.reshape(NC, NPC)
    rank = np.argsort(-deg_c, axis=1, kind='stable')
    node_ids = rank + (np.arange(NC)[:, None] * NPC)
    deg_sorted = np.take_along_axis(deg_c, rank, axis=1)

    pad = ROWS - NPC
    node_ids_p = np.concatenate(
        [node_ids, np.repeat(np.arange(NC)[:, None] * NPC, pad, axis=1)], axis=1)
    deg_p = np.concatenate([deg_sorted, np.zeros((NC, pad), np.int64)], axis=1)

    # per-tile slot width, shared across cores (SPMD: one program for all)
    K_t = np.maximum(deg_p.reshape(NC, TILES, P).max(axis=(0, 2)), 1).astype(int)

    Kmax = int(K_t.max())
    ar = np.arange(Kmax)
    slot_idx = starts[node_ids_p][:, :, None] + ar[None, None, :]
    valid = ar[None, None, :] < deg_p[:, :, None]
    ell = np.where(valid, ssrc[np.minimum(slot_idx, E - 1)],
                   node_ids_p[:, :, None]).astype(np.int32)

    flat_ell = np.concatenate(
        [ell[:, t * P:(t + 1) * P, :K_t[t]].reshape(NC, -1) for t in range(TILES)],
        axis=1)
    cntf = deg_p.astype(np.float32)

    in_maps = []
    for c in range(NC):
        in_maps.append({
            "pos": pos,
            "ell": flat_ell[c],
            "nid": node_ids_p[c].astype(np.int32).reshape(-1, 1),
            "cntf": cntf[c].reshape(-1, 1),
        })
    meta = dict(N=N, NPC=NPC, TILES=TILES, ROWS=ROWS,
                K_t=K_t, S_total=int(flat_ell.shape[1]), node_ids=node_ids)
    return in_maps, meta


def _build_program(meta):
    N, TILES, ROWS, K_t, S_total = (meta["N"], meta["TILES"], meta["ROWS"],
                                    meta["K_t"], meta["S_total"])
    nc = bass.Bass()
    pos = nc.declare_dram_parameter("pos", [N, 1], mybir.dt.float32, isOutput=False)
    ell = nc.declare_dram_parameter("ell", [S_total], mybir.dt.int32, isOutput=False)
    nid = nc.declare_dram_parameter("nid", [ROWS, 1], mybir.dt.int32, isOutput=False)
    cntf = nc.declare_dram_parameter("cntf", [ROWS, 1], mybir.dt.float32,
                                     isOutput=False)
    out = nc.declare_dram_parameter("out", [ROWS, 2], mybir.dt.float32, isOutput=True)

    with TileContext(nc) as tc:
        with (
            tc.tile_pool(name="idxp", bufs=3) as idxp,
            tc.tile_pool(name="gp", bufs=3) as gp,
            tc.tile_pool(name="smallp", bufs=4) as smallp,
        ):
            off = 0
            for t in range(TILES):
                K = int(K_t[t])
                r0 = t * P
                idx_t = idxp.tile([P, K], mybir.dt.int32, tag="idx")
                nc.sync.dma_start(
                    out=idx_t[:],
                    in_=ell[off:off + P * K].rearrange("(p k) -> p k", p=P))
                nid_t = smallp.tile([P, 1], mybir.dt.int32, tag="nid")
                nc.sync.dma_start(out=nid_t[:], in_=nid[r0:r0 + P])
                cnt_t = smallp.tile([P, 1], mybir.dt.float32, tag="cnt")
                nc.sync.dma_start(out=cnt_t[:], in_=cntf[r0:r0 + P])

                posd = smallp.tile([P, 1], mybir.dt.float32, tag="posd")
                nc.gpsimd.indirect_dma_start(
                    out=posd[:], out_offset=None, in_=pos[:],
                    in_offset=bass.IndirectOffsetOnAxis(ap=nid_t[:], axis=0))

                g_t = gp.tile([P, K], mybir.dt.float32, tag="g")
                for k in range(K):
                    nc.gpsimd.indirect_dma_start(
                        out=g_t[:, k:k + 1], out_offset=None, in_=pos[:],
                        in_offset=bass.IndirectOffsetOnAxis(
                            ap=idx_t[:, k:k + 1], axis=0))

                nc.vector.tensor_scalar(
                    out=g_t[:], in0=g_t[:], scalar1=posd[:], scalar2=None,
                    op0=mybir.AluOpType.subtract)
                s_t = smallp.tile([P, 1], mybir.dt.float32, tag="s")
                nc.vector.tensor_reduce(
                    out=s_t[:], in_=g_t[:], axis=mybir.AxisListType.X,
                    op=mybir.AluOpType.add, apply_absolute_value=True)

                nc.vector.tensor_scalar_max(out=cnt_t[:], in0=cnt_t[:], scalar1=1.0)
                nc.vector.reciprocal(out=cnt_t[:], in_=cnt_t[:])
                o_t = smallp.tile([P, 2], mybir.dt.float32, tag="o")
                nc.vector.tensor_copy(out=o_t[:, 0:1], in_=posd[:])
                nc.vector.tensor_tensor(
                    out=o_t[:, 1:2], in0=s_t[:], in1=cnt_t[:],
                    op=mybir.AluOpType.mult)
                nc.sync.dma_start(out=out[r0:r0 + P], in_=o_t[:])
                off += P * K

    _split_sync_waits(nc)
    return nc


def kernel(h, src, dst):
    h = np.asarray(h)
    src = np.asarray(src)
    dst = np.asarray(dst)
    in_maps, meta = _host_prep(h, src, dst)
    nc = _build_program(meta)
    res = run_bass_kernel_spmd(nc, in_maps, list(range(NC)))
    N, NPC, node_ids = meta["N"], meta["NPC"], meta["node_ids"]
    final = np.empty((N, 2), np.float32)
    for c in range(NC):
        final[node_ids[c]] = res.results[c]["out"][:NPC]
    return final
